# revision 21
# baseline (speedup 1.0000x reference)
"""Trainium2 Bass kernel for nn_ActorCritic (GAT actor-critic over B=16384
fully-connected 6-node graphs), data-parallel over 8 NeuronCores.

Self-contained: hardcodes all shapes; imports only the system concourse repo.
"""
import sys

if "/opt/trn_rl_repo" not in sys.path:
    sys.path.insert(0, "/opt/trn_rl_repo")

import numpy as np
import ml_dtypes

import concourse.bass as bass
import concourse.mybir as mybir
import concourse.tile as tile
from concourse import bacc

BF16 = mybir.dt.bfloat16
F32 = mybir.dt.float32
I16 = mybir.dt.int16

# problem constants
B, A, DOBS, D, H, HID, NACT = 16384, 6, 128, 250, 3, 256, 5
SLOPE = 0.2
NCORES = 8
G_PC = B // NCORES            # 2048 real graphs / core
GSUP = 84                     # graphs per super-block
SUPER = GSUP * A              # 504 nodes per super-block
NSUP_FULL = 25                # supers per core (pad 2048 -> 2100 graphs)
CHUNK = 126                   # nodes per aggregation chunk (21 graphs)
NCHUNK = SUPER // CHUNK       # 4
DIST = 6                      # pipeline distance between phase A and phase B
XT_PREF = 2                   # xt prefetch distance (supers)


def build(nsup=NSUP_FULL):
    """Trace + compile the per-core Bass graph. Returns compiled Bacc."""
    npc = nsup * SUPER  # padded nodes per core
    nc = bacc.Bacc("TRN2", target_bir_lowering=False, debug=False,
                   num_devices=NCORES)

    # ---- DRAM parameters ----
    xt_ext = nc.dram_tensor("xt", [DOBS, npc + 2], BF16, kind="ExternalInput")
    wemb_ext = nc.dram_tensor("wemb", [DOBS, 256], BF16, kind="ExternalInput")
    wg_ext = nc.dram_tensor("wg", [H, 2, 128, D], BF16, kind="ExternalInput")
    vlr_ext = nc.dram_tensor("vlr", [2, 128, 6], BF16, kind="ExternalInput")
    w1_ext = nc.dram_tensor("w1", [4, 128, 256], BF16, kind="ExternalInput")
    v1_ext = nc.dram_tensor("v1", [4, 128, 256], BF16, kind="ExternalInput")
    w2_ext = nc.dram_tensor("w2", [2, 128, NACT], BF16, kind="ExternalInput")
    v2_ext = nc.dram_tensor("v2", [2, 128, 1], BF16, kind="ExternalInput")
    bias_ext = nc.dram_tensor("biases", [128, 8], F32, kind="ExternalInput")
    b2c_ext = nc.dram_tensor("b2cat", [128, 24], F32, kind="ExternalInput")
    sidx_ext = nc.dram_tensor("sidx", [128, 80], I16, kind="ExternalInput")
    out_ext = nc.dram_tensor("out", [npc, 6], F32, kind="ExternalOutput")

    # DRAM scratch for cross-layout reshapes (per-super slots)
    elrd = nc.dram_tensor("elrd", [nsup, 6, SUPER], F32)
    natd = nc.dram_tensor("natd", [nsup, GSUP, 108], BF16)

    with tile.TileContext(nc) as tc:
        # ---- persistent SBUF: weights/constants ----
        wemb_sb = nc.alloc_sbuf_tensor("wemb_sb", [DOBS, 256], BF16)
        wg_sb = [[nc.alloc_sbuf_tensor(f"wg_sb_{h}_{k}", [128, D], BF16)
                  for k in range(2)] for h in range(H)]
        vlr_sb = [nc.alloc_sbuf_tensor(f"vlr_sb_{k}", [128, 6], BF16)
                  for k in range(2)]
        w1_sb = [nc.alloc_sbuf_tensor(f"w1_sb_{k}", [128, 256], BF16)
                 for k in range(4)]
        v1_sb = [nc.alloc_sbuf_tensor(f"v1_sb_{k}", [128, 256], BF16)
                 for k in range(4)]
        w2_sb = [nc.alloc_sbuf_tensor(f"w2_sb_{k}", [128, NACT], BF16)
                 for k in range(2)]
        v2_sb = [nc.alloc_sbuf_tensor(f"v2_sb_{k}", [128, 1], BF16)
                 for k in range(2)]
        bias_sb = nc.alloc_sbuf_tensor("bias_sb", [128, 8], F32)
        b2c_sb = nc.alloc_sbuf_tensor("b2c_sb", [128, 24], F32)
        sidx_sb = nc.alloc_sbuf_tensor("sidx_sb", [128, 80], I16)

        nc.scalar.dma_start(wemb_sb[:], wemb_ext[:])
        nc.scalar.dma_start(bias_sb[:], bias_ext[:])
        for k in range(2):
            nc.scalar.dma_start(vlr_sb[k][:], vlr_ext[k])
        nc.scalar.dma_start(sidx_sb[:], sidx_ext[:])

        # Remaining preloads are pumped at the END of the first phase_a calls
        # (behind that iteration's chain hops on each queue) — all emitted
        # before phase_b(0) at iteration DIST, so Tile's emission-order
        # dependency tracking still sees writer-before-reader.
        deferred = [(wg_sb[h][k][:], wg_ext[h, k])
                    for h in range(H) for k in range(2)]
        deferred += [(w1_sb[k][:], w1_ext[k]) for k in range(4)]
        deferred += [(v1_sb[k][:], v1_ext[k]) for k in range(4)]
        deferred += [(w2_sb[k][:], w2_ext[k]) for k in range(2)]
        deferred += [(v2_sb[k][:], v2_ext[k]) for k in range(2)]
        deferred += [(b2c_sb[:], b2c_ext[:])]
        _pump_i = [0]

        def pump_preloads(n):
            engines = [nc.sync, nc.scalar, nc.gpsimd]
            for _ in range(n):
                if not deferred:
                    return
                dst, srcd = deferred.pop(0)
                engines[_pump_i[0] % 3].dma_start(dst, srcd)
                _pump_i[0] += 1

        # ---- pools ----
        import contextlib
        ctx = contextlib.ExitStack()
        with ctx:
            xpool = ctx.enter_context(tc.tile_pool(name="xp", bufs=4))
            hpool = ctx.enter_context(tc.tile_pool(name="hp", bufs=2 * (DIST + 1)))
            gpool = ctx.enter_context(tc.tile_pool(name="gp", bufs=2))
            whpool = ctx.enter_context(tc.tile_pool(name="whp", bufs=4))
            a1pool = ctx.enter_context(tc.tile_pool(name="a1p", bufs=8))
            apool = ctx.enter_context(tc.tile_pool(name="ap", bufs=DIST + 1))
            spool = ctx.enter_context(tc.tile_pool(name="sp", bufs=4))
            opool = ctx.enter_context(tc.tile_pool(name="op", bufs=3))
            # PSUM: 8 banks total; tags: emb 2 + elr 1 + big2 3 + gat 2
            ppool = ctx.enter_context(tc.tile_pool(name="pp", bufs=2, space="PSUM"))

            xt_tiles = {}

            def load_xt(s):
                n0 = s * SUPER
                xt = xpool.tile([DOBS, SUPER + 2], BF16, tag="xt")
                nc.sync.dma_start(xt[:], xt_ext[:, n0:n0 + SUPER + 2])
                xt_tiles[s] = xt

            def phase_a(s):
                if s + XT_PREF < nsup:
                    load_xt(s + XT_PREF)
                xt = xt_tiles.pop(s)
                # ---------- embed ----------
                h_ps0 = ppool.tile([128, SUPER + 2], F32, tag="emb", bufs=2)
                h_ps1 = ppool.tile([128, SUPER + 2], F32, tag="emb", bufs=2)
                nc.tensor.matmul(h_ps0[:], wemb_sb[:, 0:128], xt[:],
                                 start=True, stop=True)
                nc.tensor.matmul(h_ps1[:], wemb_sb[:, 128:256], xt[:],
                                 start=True, stop=True)
                hT0 = hpool.tile([128, SUPER + 2], BF16, tag="hT")
                hT1 = hpool.tile([128, SUPER + 2], BF16, tag="hT")
                nc.vector.tensor_scalar(
                    hT0[:, 0:506], h_ps0[:, 0:506],
                    bias_sb[:, 0:1], 0.0,
                    mybir.AluOpType.add, mybir.AluOpType.max)
                nc.scalar.activation(
                    hT1[:, 0:506], h_ps1[:, 0:506],
                    mybir.ActivationFunctionType.Relu,
                    bias=bias_sb[:, 1:2])
                # ---------- el/er ----------
                elr_ps = ppool.tile([6, SUPER + 2], F32, tag="elr", bufs=1)
                nc.tensor.matmul(elr_ps[:], vlr_sb[0][:], hT0[:],
                                 start=True, stop=False)
                nc.tensor.matmul(elr_ps[:], vlr_sb[1][:], hT1[:],
                                 start=False, stop=True)
                elr_sb = spool.tile([6, SUPER], F32, tag="elr_sb")
                nc.scalar.copy(elr_sb[:], elr_ps[:, 0:SUPER])
                # transpose to graph-major via DRAM bounce (sync hwdge queue)
                nc.sync.dma_start(elrd[s], elr_sb[:])
                elrg = spool.tile([GSUP, 36], F32, tag="elrg")
                src = bass.AP(tensor=elrd[:].tensor, offset=s * 6 * SUPER,
                              ap=[[6, GSUP], [SUPER, 6], [1, 6]])
                nc.sync.dma_start(elrg[:].rearrange("p (r i) -> p r i", i=6), src)
                # ---------- attention stats (graph-major) ----------
                e_t = spool.tile([GSUP, 108], F32, tag="e_t")
                in0 = (elrg[:, 0:18].rearrange("p (h i) -> p h i", i=6)
                       .unsqueeze(3).broadcast_to((GSUP, 3, 6, 6)))
                in1 = (elrg[:, 18:36].rearrange("p (h j) -> p h j", j=6)
                       .unsqueeze(2).broadcast_to((GSUP, 3, 6, 6)))
                ev = e_t[:].rearrange("p (h i j) -> p h i j", i=6, j=6)
                nc.vector.tensor_tensor(ev, in0, in1, mybir.AluOpType.add)
                w_t = spool.tile([GSUP, 108], F32, tag="w_t")
                lk = spool.tile([GSUP, 108], F32, tag="lk")
                nc.vector.scalar_tensor_tensor(lk[:], e_t[:], SLOPE, e_t[:],
                                               mybir.AluOpType.mult,
                                               mybir.AluOpType.max)
                nc.scalar.activation(w_t[:], lk[:],
                                     mybir.ActivationFunctionType.Exp)
                s_t = spool.tile([GSUP, 18], F32, tag="s_t")
                nc.vector.tensor_reduce(
                    s_t[:], w_t[:].rearrange("p (hi j) -> p hi j", j=6),
                    mybir.AxisListType.X, mybir.AluOpType.add)
                r_t = spool.tile([GSUP, 18], F32, tag="r_t")
                nc.vector.reciprocal(r_t[:], s_t[:])
                natg = spool.tile([GSUP, 112], BF16, tag="natg")
                nout = natg[:, 0:108].rearrange("p (j h i) -> p h i j", h=3, i=6)
                nin0 = w_t[:].rearrange("p (h i j) -> p h i j", i=6, j=6)
                nin1 = (r_t[:].rearrange("p (h i) -> p h i", i=6)
                        .unsqueeze(3).broadcast_to((GSUP, 3, 6, 6)))
                nc.vector.tensor_tensor(nout, nin0, nin1, mybir.AluOpType.mult)
                nc.gpsimd.dma_start(natd[s], natg[:, 0:108])
                # ---------- node-major gather + block-diag scatter ----------
                dataN4 = spool.tile([128, 80], BF16, tag="dataN")
                for k in range(NCHUNK):
                    srcn = bass.AP(tensor=natd[:].tensor,
                                   offset=(s * GSUP + 21 * k) * 108,
                                   ap=[[108, 21], [18, 6], [1, 18]])
                    eng = nc.sync if k % 2 == 0 else nc.gpsimd
                    eng.dma_start(dataN4[0:CHUNK, 20 * k:20 * k + 18], srcn)
                at4 = apool.tile([128, 4 * 384], BF16, tag="atall")
                nc.gpsimd.local_scatter(at4[:], dataN4[:], sidx_sb[:],
                                        channels=128, num_elems=4 * 384,
                                        num_idxs=80)
                pump_preloads(5)
                return (s, hT0, hT1, at4)

            def phase_b(state):
                s, hT0, hT1, at4 = state
                n0 = s * SUPER
                # ---------- per-chunk: aug Wh + aggregation ----------
                # wh for chunk k is computed one step ahead of gat for chunk k
                # so the PSUM->SBUF cast latency hides under PE work.
                g0 = gpool.tile([128, SUPER + 2], BF16, tag="g0")
                g1 = gpool.tile([128, SUPER + 2], BF16, tag="g1")
                gatL = ppool.tile([128, SUPER], F32, tag="gat", bufs=2)
                gatU = ppool.tile([128, SUPER], F32, tag="gat", bufs=2)
                wh_tiles = [None] * NCHUNK

                def wh_chunk(k):
                    c0 = k * CHUNK
                    wh_ps01 = ppool.tile([128, 2 * D], F32, tag="big2", bufs=3)
                    for h in range(2):
                        nc.tensor.matmul(wh_ps01[:, h * D:h * D + D],
                                         hT0[:, c0:c0 + 128], wg_sb[h][0][:],
                                         start=True, stop=False)
                        nc.tensor.matmul(wh_ps01[:, h * D:h * D + D],
                                         hT1[:, c0:c0 + 128], wg_sb[h][1][:],
                                         start=False, stop=True)
                    wh_ps2 = ppool.tile([128, 2 * D], F32, tag="big2", bufs=3)
                    nc.tensor.matmul(wh_ps2[:, 0:D],
                                     hT0[:, c0:c0 + 128], wg_sb[2][0][:],
                                     start=True, stop=False)
                    nc.tensor.matmul(wh_ps2[:, 0:D],
                                     hT1[:, c0:c0 + 128], wg_sb[2][1][:],
                                     start=False, stop=True)
                    wsb01 = whpool.tile([128, 2 * D], BF16, tag="wh01")
                    nc.vector.tensor_copy(wsb01[0:CHUNK, :], wh_ps01[0:CHUNK, :])
                    wsb2 = whpool.tile([128, D], BF16, tag="wh2")
                    nc.scalar.copy(wsb2[0:CHUNK, :], wh_ps2[0:CHUNK, 0:D])
                    wh_tiles[k] = (wsb01, wsb2)

                def gat_chunk(k):
                    c0 = k * CHUNK
                    wsb01, wsb2 = wh_tiles[k]
                    wh_t = [(wsb01, 0), (wsb01, D), (wsb2, 0)]
                    for h in range(H):
                        tl, off = wh_t[h]
                        nc.tensor.matmul(
                            gatL[:, c0:c0 + CHUNK],
                            tl[0:CHUNK, off:off + 128],
                            at4[0:CHUNK, 384 * k + 128 * h:384 * k + 128 * h + CHUNK],
                            start=(h == 0), stop=(h == 2))
                    for h in range(H):
                        tl, off = wh_t[h]
                        nc.tensor.matmul(
                            gatU[0:122, c0:c0 + CHUNK],
                            tl[0:CHUNK, off + 128:off + D],
                            at4[0:CHUNK, 384 * k + 128 * h:384 * k + 128 * h + CHUNK],
                            start=(h == 0), stop=(h == 2))

                wh_chunk(0)
                wh_chunk(1)
                gat_chunk(0)
                wh_chunk(2)
                gat_chunk(1)
                wh_chunk(3)
                gat_chunk(2)
                gat_chunk(3)
                nc.vector.tensor_copy(g0[:, 0:SUPER], gatL[:, 0:SUPER])
                nc.scalar.copy(g1[0:122, 0:SUPER], gatU[0:122, 0:SUPER])

                # ---------- heads ----------
                feat_tiles = [hT0, hT1, g0, g1]
                a1_sb, c1_sb = [], []
                for which, (wsb, bcol, dst) in enumerate(
                        [(w1_sb, 2, a1_sb), (v1_sb, 4, c1_sb)]):
                    for m in range(2):
                        ps = ppool.tile([128, SUPER + 2], F32, tag="big2", bufs=3)
                        for kt in range(4):
                            kk = 122 if kt == 3 else 128
                            nc.tensor.matmul(ps[:],
                                             wsb[kt][0:kk, 128 * m:128 * m + 128],
                                             feat_tiles[kt][0:kk, :],
                                             start=(kt == 0), stop=(kt == 3))
                        sb = a1pool.tile([128, SUPER + 2], BF16, tag="a1")
                        bap = bias_sb[:, bcol + m:bcol + m + 1]
                        if (which, m) == (0, 0):
                            nc.vector.tensor_scalar(
                                sb[:, 0:506], ps[:, 0:506], bap, 0.0,
                                mybir.AluOpType.add, mybir.AluOpType.max)
                        else:
                            nc.scalar.activation(
                                sb[:, 0:506], ps[:, 0:506],
                                mybir.ActivationFunctionType.Relu, bias=bap)
                        dst.append(sb)
                zn_ps = ppool.tile([128, 24], F32, tag="gat", bufs=2)
                for k in range(NCHUNK):
                    c0 = k * CHUNK
                    for kt in range(2):
                        nc.tensor.matmul(zn_ps[:, 6 * k:6 * k + NACT],
                                         a1_sb[kt][:, c0:c0 + 128],
                                         w2_sb[kt][:],
                                         start=(kt == 0), stop=(kt == 1))
                    for kt in range(2):
                        nc.tensor.matmul(zn_ps[:, 6 * k + NACT:6 * k + 6],
                                         c1_sb[kt][:, c0:c0 + 128],
                                         v2_sb[kt][:],
                                         start=(kt == 0), stop=(kt == 1))
                # ---------- final softmax + output ----------
                pst = spool.tile([CHUNK, 24], F32, tag="pst")
                nc.vector.tensor_add(pst[:], zn_ps[0:CHUNK, :], b2c_sb[0:CHUNK, :])
                wst = spool.tile([CHUNK, 20], F32, tag="wst")
                pin = pst[:].rearrange("p (k c) -> p k c", c=6)[:, :, 0:NACT]
                nc.scalar.activation(wst[:].rearrange("p (k a) -> p k a", a=5),
                                     pin, mybir.ActivationFunctionType.Exp)
                s4 = spool.tile([CHUNK, 4], F32, tag="s4")
                nc.vector.tensor_reduce(
                    s4[:], wst[:].rearrange("p (k a) -> p k a", a=5),
                    mybir.AxisListType.X, mybir.AluOpType.add)
                r4 = spool.tile([CHUNK, 4], F32, tag="r4")
                nc.vector.reciprocal(r4[:], s4[:])
                outst = opool.tile([CHUNK, 24], F32, tag="outst")
                oview = outst[:].rearrange("p (k c) -> p k c", c=6)
                nc.vector.tensor_tensor(
                    oview[:, :, 0:NACT],
                    wst[:].rearrange("p (k a) -> p k a", a=5),
                    r4[:].unsqueeze(2).broadcast_to((CHUNK, 4, NACT)),
                    mybir.AluOpType.mult)
                nc.gpsimd.tensor_copy(oview[:, :, NACT:6],
                                      pst[:].rearrange("p (k c) -> p k c", c=6)
                                      [:, :, NACT:6])
                dst = bass.AP(tensor=out_ext[:].tensor, offset=n0 * 6,
                              ap=[[6, CHUNK], [CHUNK * 6, NCHUNK], [1, 6]])
                nc.gpsimd.dma_start(
                    dst, outst[:].rearrange("p (k c) -> p k c", c=6))

            # ---- main pipeline: B(s-DIST) before A(s) each iteration ----
            load_xt(0)
            if XT_PREF > 1 and nsup > 1:
                load_xt(1)
            q = []
            for s in range(nsup + DIST):
                if s >= DIST:
                    phase_b(q.pop(0))
                if s < nsup:
                    q.append(phase_a(s))

    nc.compile()
    return nc


def prepare_inputs(x, W_emb, b_emb, W_gat, a_l, a_r, W1, b1, W2, b2,
                   V1, vb1, V2, vb2, nsup=NSUP_FULL):
    """Host-side: shard/pad/transpose x; precompute packed weights."""
    npc = nsup * SUPER
    bf = ml_dtypes.bfloat16
    x = np.asarray(x, np.float32)
    n_real_pc = x.shape[0] // NCORES

    wemb = np.zeros((DOBS, 256), np.float32)
    wemb[:, :D] = np.asarray(W_emb)
    wg = np.zeros((H, 2, 128, D), np.float32)
    Wg = np.asarray(W_gat)
    for h in range(H):
        pad = np.zeros((256, D), np.float32)
        pad[:D] = Wg[h]
        wg[h, 0] = pad[0:128]
        wg[h, 1] = pad[128:256]
    vlr = np.zeros((2, 128, 6), np.float32)
    for h in range(H):
        vl = Wg[h] @ np.asarray(a_l)[h]
        vr = Wg[h] @ np.asarray(a_r)[h]
        vlr[0, :, h] = np.pad(vl, (0, 6))[0:128]
        vlr[1, :, h] = np.pad(vl, (0, 6))[128:256]
        vlr[0, :, 3 + h] = np.pad(vr, (0, 6))[0:128]
        vlr[1, :, 3 + h] = np.pad(vr, (0, 6))[128:256]
    # W1_eff rows: [0:250]=W1 top, [250:256]=0, [256:506]=W1 bottom / 3
    def pack_head1(Wm):
        Wm = np.asarray(Wm)
        eff = np.zeros((512, 256), np.float32)
        eff[0:D] = Wm[0:D]
        eff[256:256 + D] = Wm[D:2 * D] / 3.0
        return np.stack([eff[128 * k:128 * k + 128] for k in range(4)])
    w1 = pack_head1(W1)
    v1 = pack_head1(V1)
    w2 = np.stack([np.asarray(W2)[0:128], np.asarray(W2)[128:256]])
    v2 = np.stack([np.asarray(V2)[0:128], np.asarray(V2)[128:256]])
    biases = np.zeros((128, 8), np.float32)
    be = np.pad(np.asarray(b_emb), (0, 6))
    biases[:, 0] = be[0:128]
    biases[:, 1] = be[128:256]
    biases[:, 2] = np.asarray(b1)[0:128]
    biases[:, 3] = np.asarray(b1)[128:256]
    biases[:, 4] = np.asarray(vb1)[0:128]
    biases[:, 5] = np.asarray(vb1)[128:256]
    b2cat = np.zeros((128, 24), np.float32)
    for k in range(4):
        b2cat[:, 6 * k:6 * k + 5] = np.asarray(b2)[None, :]
        b2cat[:, 6 * k + 5] = np.asarray(vb2)[0]
    sidx = np.full((128, 80), -1, np.int16)
    for p in range(CHUNK):
        gg, j = p // 6, p % 6
        for k in range(4):
            for h in range(H):
                for i in range(6):
                    sidx[p, 20 * k + h * 6 + i] = 384 * k + 128 * h + 6 * gg + i

    shared = {
        "wemb": wemb.astype(bf), "wg": wg.astype(bf), "vlr": vlr.astype(bf),
        "w1": w1.astype(bf), "v1": v1.astype(bf), "w2": w2.astype(bf),
        "v2": v2.astype(bf), "biases": biases, "b2cat": b2cat, "sidx": sidx,
    }
    in_maps = []
    for c in range(NCORES):
        xs = x[c * n_real_pc:(c + 1) * n_real_pc]
        xp = np.zeros((npc + 2, DOBS), np.float32)
        xp[0:min(n_real_pc, npc)] = xs[0:npc]
        m = dict(shared)
        m["xt"] = np.ascontiguousarray(xp.T).astype(bf)
        in_maps.append(m)
    return in_maps, n_real_pc


_BUILD_CACHE = {}


def _get_built(nsup):
    if nsup not in _BUILD_CACHE:
        _BUILD_CACHE[nsup] = build(nsup)
    return _BUILD_CACHE[nsup]


def kernel(**inputs) -> np.ndarray:
    from concourse.bass_utils import run_bass_kernel_spmd
    nc = _get_built(NSUP_FULL)
    in_maps, n_real_pc = prepare_inputs(**inputs)
    res = run_bass_kernel_spmd(nc, in_maps, core_ids=list(range(NCORES)),
                               trace=False)
    outs = [res.results[c]["out"][0:n_real_pc] for c in range(NCORES)]
    return np.concatenate(outs, axis=0).astype(np.float32)
